# revision 10
# baseline (speedup 1.0000x reference)
"""Two-layer GAT (PyG GATConv semantics) on 8 Trainium2 NeuronCores.

Strategy (node/graph parallel):
- Nodes are degree-sorted and dealt round-robin to 8 cores (dst sharding).
- Every core computes the full [h | a_src] projection tables for both layers
  (replicated compute beats all-gathering the 100MB layer-2 table).
- Per-edge work is dst-aligned: for each group of 128 destination nodes
  (one per SBUF partition), padded per-lane slot lists drive [128,1]-indexed
  row gathers from the projection table; attention softmax runs without
  max-subtraction (logits are O(10), fp32 exp is safe; the 1e-16 eps makes
  the normalization exactly alpha = ex / (sum ex + eps) either way).
- Sentinel table row (a_src = -1000, h = 0) makes padded slots contribute
  exactly zero weight.
- Single AllGather exchanges the transposed layer-1 outputs (the only
  cross-core communication).
"""
import sys

sys.path.insert(0, "/opt/trn_rl_repo")

from contextlib import ExitStack

import numpy as np

import concourse.bass as bass
import concourse.tile as tile
from concourse import mybir
import bass_rust as _bass_rust
from concourse.bass_utils import run_bass_kernel_spmd
from concourse.masks import make_identity

NC = 8
P = 128
HEADS = 4
NEG_SLOPE = 0.2
EPS = 1e-16
SENT_ASRC = -1000.0

F32 = mybir.dt.float32
I32 = mybir.dt.int32


def _host_prep(x, edge_index):
    n, in_ch = x.shape
    src = np.concatenate([np.asarray(edge_index[0]), np.arange(n, dtype=np.int64)])
    dst = np.concatenate([np.asarray(edge_index[1]), np.arange(n, dtype=np.int64)])
    deg = np.bincount(dst, minlength=n)

    order = np.argsort(-deg, kind="stable")  # order[rank] = node
    rank = np.empty(n, dtype=np.int64)
    rank[order] = np.arange(n)

    nloc = ((n + NC - 1) // NC + P - 1) // P * P  # local slots per core
    ntab = NC * nloc
    ng = nloc // P

    # rank r -> core r%NC, slot r//NC, table row t
    t_of = (rank % NC) * nloc + rank // NC  # per node

    x_tab = np.zeros((ntab, in_ch), dtype=np.float32)
    x_tab[t_of] = np.asarray(x, dtype=np.float32)
    x_tabT = np.ascontiguousarray(x_tab.T)  # [in_ch, ntab]

    td = t_of[dst]
    ts = t_of[src].astype(np.int32)
    c_e = td // nloc
    loc = td % nloc
    g_e = loc // P
    lane_e = loc % P

    key = (c_e * ng + g_e) * P + lane_e
    cnt = np.bincount(key, minlength=NC * ng * P).reshape(NC, ng, P)
    s_g = np.maximum(cnt.max(axis=(0, 2)), 1)  # padded slots per group
    s0 = np.zeros(ng, dtype=np.int64)
    s0[1:] = np.cumsum(s_g)[:-1]
    st = int(s_g.sum())

    sidx = np.argsort(key, kind="stable")
    ks = key[sidx]
    # slot index within each key run
    starts = np.searchsorted(ks, np.arange(NC * ng * P))
    slot = np.arange(len(ks)) - starts[ks]

    idx_arr = np.full((NC, P, st), ntab, dtype=np.int32)  # sentinel row id
    col = s0[g_e[sidx]] + slot
    idx_arr[c_e[sidx], lane_e[sidx], col] = ts[sidx]

    adstbase = np.stack(
        [(c * nloc + np.arange(P) * ng).astype(np.int32).reshape(P, 1) for c in range(NC)]
    )

    meta = {
        "n": n,
        "nloc": nloc,
        "ntab": ntab,
        "ng": ng,
        "st": st,
        "s_g": s_g.astype(np.int64),
        "s0": s0,
        "order": order,
    }
    return x_tabT, idx_arr, adstbase, meta


def _build_program(meta, in_ch, hid, out_ch):
    """One SPMD program for all 8 cores."""
    ntab, ng, st, nloc = meta["ntab"], meta["ng"], meta["st"], meta["nloc"]
    s_g, s0 = meta["s_g"], meta["s0"]
    f1 = HEADS * hid       # 64
    f2 = HEADS * out_ch    # 256
    ntiles = ntab // P

    nc = bass.Bass(num_devices=NC)

    x_tabT = nc.declare_dram_parameter("x_tabT", [in_ch, ntab], F32, isOutput=False)
    idx_d = nc.declare_dram_parameter("idx", [P, st], I32, isOutput=False)
    adstbase_d = nc.declare_dram_parameter("adstbase", [P, 1], I32, isOutput=False)
    w1_d = nc.declare_dram_parameter("w1", [in_ch, f1], F32, isOutput=False)
    asrc1_d = nc.declare_dram_parameter("asrc1", [1, f1], F32, isOutput=False)
    adst1_d = nc.declare_dram_parameter("adst1", [1, f1], F32, isOutput=False)
    b1_d = nc.declare_dram_parameter("b1", [1, f1], F32, isOutput=False)
    w2_d = nc.declare_dram_parameter("w2", [f1, f2], F32, isOutput=False)
    asrc2_d = nc.declare_dram_parameter("asrc2", [1, f2], F32, isOutput=False)
    adst2_d = nc.declare_dram_parameter("adst2", [1, f2], F32, isOutput=False)
    b2_d = nc.declare_dram_parameter("b2", [1, out_ch], F32, isOutput=False)
    out2_d = nc.declare_dram_parameter("out2", [nloc, out_ch], F32, isOutput=True)

    h1cat = nc.dram_tensor("h1cat", [ntab + 1, f1 + 4], F32)
    adstp1 = nc.dram_tensor("adstp1", [ntab, 4], F32)
    h2cat = nc.dram_tensor("h2cat", [ntab + 1, f2 + 4], F32)
    adstp2 = nc.dram_tensor("adstp2", [ntab, 4], F32)
    out1t = nc.dram_tensor("out1t", [f1, nloc], F32)
    ag_out = nc.dram_tensor("ag_out", [NC * f1, nloc], F32, addr_space="Shared")

    def bcast_row(dram_t, width):
        return bass.AP(
            tensor=dram_t[:].tensor,
            offset=dram_t[:].offset,
            ap=[[0, P], [1, width]],
        )

    with tile.TileContext(nc) as tc, ExitStack() as ctx:
        const = ctx.enter_context(tc.tile_pool(name="const", bufs=1))
        pool = ctx.enter_context(tc.tile_pool(name="work", bufs=3))
        gpool = ctx.enter_context(tc.tile_pool(name="gather", bufs=2))
        psum = ctx.enter_context(tc.tile_pool(name="psum", bufs=2, space="PSUM"))
        psum1 = ctx.enter_context(tc.tile_pool(name="psum1", bufs=1, space="PSUM"))

        # ---- constants ----
        w1_sb = const.tile([in_ch, f1], F32)
        nc.sync.dma_start(out=w1_sb[:], in_=w1_d[:, :])
        w2_sb = const.tile([f1, f2], F32)
        nc.sync.dma_start(out=w2_sb[:], in_=w2_d[:, :])
        b1_b = const.tile([P, f1], F32)
        nc.sync.dma_start(out=b1_b[:], in_=bcast_row(b1_d, f1))
        b2_b = const.tile([P, out_ch], F32)
        nc.sync.dma_start(out=b2_b[:], in_=bcast_row(b2_d, out_ch))
        ident = const.tile([P, P], F32)
        make_identity(nc, ident[:])
        idx_sb = const.tile([P, st], I32)
        nc.sync.dma_start(out=idx_sb[:], in_=idx_d[:, :])
        adstbase_sb = const.tile([P, 1], I32)
        nc.sync.dma_start(out=adstbase_sb[:], in_=adstbase_d[:, :])

        # sentinel rows
        sent1 = const.tile([1, f1 + 4], F32)
        nc.vector.memset(sent1[:], 0.0)
        nc.vector.memset(sent1[:, f1 : f1 + 4], SENT_ASRC)
        nc.sync.dma_start(out=h1cat[ntab : ntab + 1, :], in_=sent1[:])
        sent2 = const.tile([1, f2 + 4], F32)
        nc.vector.memset(sent2[:], 0.0)
        nc.vector.memset(sent2[:, f2 : f2 + 4], SENT_ASRC)
        nc.sync.dma_start(out=h2cat[ntab : ntab + 1, :], in_=sent2[:])

        # ---- augmented weights: Waug = [W | W @ Asrc_bd | W @ Adst_bd] ----
        # Block-diagonal attention matrices A[h*ch + c, h] = att[h, c], built by
        # DMA'ing each head's att row into a column block.
        def build_attmat(att_d, fdim, tag):
            # returns per-128-row chunks of the [fdim, 4] block-diag matrix
            ch = fdim // HEADS
            chunks = []
            for k0 in range(0, fdim, P):
                rows = min(P, fdim - k0)
                a_sb = const.tile([rows, 4], F32, tag=f"{tag}_{k0}")
                nc.vector.memset(a_sb[:], 0.0)
                for h in range(HEADS):
                    lo, hi = h * ch, (h + 1) * ch
                    lo2, hi2 = max(lo, k0), min(hi, k0 + rows)
                    if lo2 < hi2:
                        nc.sync.dma_start(
                            out=a_sb[lo2 - k0 : hi2 - k0, h : h + 1],
                            in_=att_d[0:1, lo2:hi2],
                        )
                chunks.append(a_sb)
            return chunks

        as1_m = build_attmat(asrc1_d, f1, "as1m")
        ad1_m = build_attmat(adst1_d, f1, "ad1m")
        as2_m = build_attmat(asrc2_d, f2, "as2m")
        ad2_m = build_attmat(adst2_d, f2, "ad2m")

        # W1 @ A (contraction over f1=64): lhsT = W1^T via PE transpose.
        w1aug = const.tile([in_ch, f1 + 8], F32)
        nc.vector.tensor_copy(out=w1aug[:, 0:f1], in_=w1_sb[:])
        w1t_ps = psum1.tile([f1, in_ch], F32, tag="prep_t")
        nc.tensor.transpose(out=w1t_ps[:], in_=w1_sb[:], identity=ident[:])
        w1t = const.tile([f1, in_ch], F32)
        nc.vector.tensor_copy(out=w1t[:], in_=w1t_ps[:])
        w1as_ps = psum1.tile([in_ch, 4], F32, tag="prep_a")
        w1ad_ps = psum1.tile([in_ch, 4], F32, tag="prep_b")
        nc.tensor.matmul(out=w1as_ps[:], lhsT=w1t[:], rhs=as1_m[0][:], start=True, stop=True)
        nc.tensor.matmul(out=w1ad_ps[:], lhsT=w1t[:], rhs=ad1_m[0][:], start=True, stop=True)
        nc.vector.tensor_copy(out=w1aug[:, f1 : f1 + 4], in_=w1as_ps[:])
        nc.vector.tensor_copy(out=w1aug[:, f1 + 4 : f1 + 8], in_=w1ad_ps[:])

        # W2 @ A2 (contraction over f2=256, split into K=128 halves).
        w2aug = const.tile([f1, f2 + 8], F32)
        nc.vector.tensor_copy(out=w2aug[:, 0:f2], in_=w2_sb[:])
        w2as_ps = psum1.tile([f1, 4], F32, tag="prep_a")
        w2ad_ps = psum1.tile([f1, 4], F32, tag="prep_b")
        nkh = f2 // P
        for kh in range(nkh):
            w2t_ps = psum1.tile([P, f1], F32, tag="prep_t")
            nc.tensor.transpose(
                out=w2t_ps[:], in_=w2_sb[:, kh * P : (kh + 1) * P],
                identity=ident[0:f1, 0:f1],
            )
            w2t = pool.tile([P, f1], F32, tag="w2t_sb")
            nc.vector.tensor_copy(out=w2t[:], in_=w2t_ps[:])
            nc.tensor.matmul(
                out=w2as_ps[:], lhsT=w2t[:], rhs=as2_m[kh][:],
                start=(kh == 0), stop=(kh == nkh - 1),
            )
            nc.tensor.matmul(
                out=w2ad_ps[:], lhsT=w2t[:], rhs=ad2_m[kh][:],
                start=(kh == 0), stop=(kh == nkh - 1),
            )
        nc.vector.tensor_copy(out=w2aug[:, f2 : f2 + 4], in_=w2as_ps[:])
        nc.vector.tensor_copy(out=w2aug[:, f2 + 4 : f2 + 8], in_=w2ad_ps[:])

        def projection(t_i, lhsT_ap, waug_sb, fdim, hcat, adstp):
            ps = psum.tile([P, fdim + 8], F32, tag="proj_ps")
            nc.tensor.matmul(out=ps[:], lhsT=lhsT_ap, rhs=waug_sb[:], start=True, stop=True)
            hc = pool.tile([P, fdim + 8], F32, tag="proj_hc")
            nc.vector.tensor_copy(out=hc[:], in_=ps[:])
            nc.sync.dma_start(
                out=hcat[t_i * P : (t_i + 1) * P, :], in_=hc[:, 0 : fdim + 4]
            )
            c_i, g_i = t_i // ng, t_i % ng
            nc.sync.dma_start(
                out=adstp[:].rearrange("(c l g) f -> c l g f", c=NC, l=P)[c_i, :, g_i, :],
                in_=hc[:, fdim + 4 : fdim + 8],
            )

        # ---- P1: layer-1 projection table (all nodes, replicated) ----
        for t_i in range(ntiles):
            xt = pool.tile([in_ch, P], F32, tag="p1_x")
            nc.sync.dma_start(out=xt[:], in_=x_tabT[:, t_i * P : (t_i + 1) * P])
            projection(t_i, xt[:], w1aug, f1, h1cat, adstp1)

        def aggregate(fdim, hcat, adstp, finish):
            """Per dst-group attention + weighted aggregation."""
            adst_own = const.tile([P, ng, 4], F32, tag=f"adst_own{fdim}")
            nc.gpsimd.indirect_dma_start(
                out=adst_own[:].rearrange("p g f -> p (g f)"),
                out_offset=None,
                in_=adstp[:],
                in_offset=bass.IndirectOffsetOnAxis(ap=adstbase_sb[:], axis=0),
            )
            smax = int(s_g.max())
            for g in range(ng):
                sg = int(s_g[g])
                base = int(s0[g])
                t_sb = gpool.tile([P, smax, fdim + 4], F32, tag=f"t{fdim}")
                for j in range(sg):
                    nc.gpsimd.indirect_dma_start(
                        out=t_sb[:, j, :],
                        out_offset=None,
                        in_=hcat[:],
                        in_offset=bass.IndirectOffsetOnAxis(
                            ap=idx_sb[:, base + j : base + j + 1], axis=0
                        ),
                    )
                w_sb = pool.tile([P, smax, 4], F32, tag=f"w{fdim}")
                den = pool.tile([P, 4], F32, tag=f"den{fdim}")
                adst_g = adst_own[:, g, :]
                adst_bc = bass.AP(
                    tensor=adst_g.tensor,
                    offset=adst_g.offset,
                    ap=[adst_g.ap[0], [0, sg], adst_g.ap[1]],
                )
                nc.vector.tensor_tensor(
                    out=w_sb[:, :sg, :],
                    in0=t_sb[:, :sg, fdim : fdim + 4],
                    in1=adst_bc,
                    op=mybir.AluOpType.add,
                )
                nc.vector.scalar_tensor_tensor(
                    out=w_sb[:, :sg, :],
                    in0=w_sb[:, :sg, :],
                    scalar=NEG_SLOPE,
                    in1=w_sb[:, :sg, :],
                    op0=mybir.AluOpType.mult,
                    op1=mybir.AluOpType.max,
                )
                for h in range(HEADS):
                    nc.scalar.activation(
                        out=w_sb[:, :sg, h],
                        in_=w_sb[:, :sg, h],
                        func=mybir.ActivationFunctionType.Exp,
                        accum_out=den[:, h : h + 1],
                    )
                hv = t_sb[:, :sg, 0:fdim].rearrange("p g (h c) -> p g h c", h=HEADS)
                nc.vector.tensor_tensor(
                    out=hv,
                    in0=hv,
                    in1=w_sb[:, :sg, :].to_broadcast([P, sg, HEADS, fdim // HEADS]),
                    op=mybir.AluOpType.mult,
                )
                u = pool.tile([P, fdim], F32, tag=f"u{fdim}")
                nc.vector.tensor_reduce(
                    out=u[:],
                    in_=t_sb[:, :sg, 0:fdim].rearrange("p g f -> p f g"),
                    axis=mybir.AxisListType.X,
                    op=mybir.AluOpType.add,
                )
                r = pool.tile([P, 4], F32, tag=f"r{fdim}")
                nc.vector.tensor_scalar_add(out=r[:], in0=den[:], scalar1=EPS)
                nc.vector.reciprocal(out=r[:], in_=r[:])
                finish(g, u, r)

        # ---- A1: layer-1 aggregation -> out1t (transposed, for AG) ----
        def finish1(g, u, r):
            o = pool.tile([P, f1], F32, tag="o1")
            nc.vector.tensor_tensor(
                out=o[:].rearrange("p (h c) -> p h c", h=HEADS),
                in0=u[:].rearrange("p (h c) -> p h c", h=HEADS),
                in1=r[:].to_broadcast([P, HEADS, f1 // HEADS]),
                op=mybir.AluOpType.mult,
            )
            nc.vector.tensor_add(out=o[:], in0=o[:], in1=b1_b[:])
            nc.vector.tensor_scalar_max(out=o[:], in0=o[:], scalar1=0.0)
            tps = psum.tile([f1, P], F32, tag="tr_ps")
            nc.tensor.transpose(out=tps[:], in_=o[:], identity=ident[:])
            ot = pool.tile([f1, P], F32, tag="o1t")
            nc.vector.tensor_copy(out=ot[:], in_=tps[:])
            nc.sync.dma_start(out=out1t[:, g * P : (g + 1) * P], in_=ot[:])

        aggregate(f1, h1cat, adstp1, finish1)

        # ---- AG: exchange layer-1 outputs ----
        nc.gpsimd.collective_compute(
            "AllGather",
            mybir.AluOpType.bypass,
            replica_groups=[list(range(NC))],
            ins=[out1t[:]],
            outs=[ag_out[:]],
        )

        # ---- P2: layer-2 projection table ----
        for t_i in range(ntiles):
            c_i, g_i = t_i // ng, t_i % ng
            lt = pool.tile([f1, P], F32, tag="p2_l")
            nc.sync.dma_start(
                out=lt[:], in_=ag_out[c_i * f1 : (c_i + 1) * f1, g_i * P : (g_i + 1) * P]
            )
            projection(t_i, lt[:], w2aug, f2, h2cat, adstp2)

        # ---- A2: layer-2 aggregation -> mean over heads + b2 ----
        def finish2(g, u, r):
            rq = pool.tile([P, 4], F32, tag="rq")
            nc.vector.tensor_scalar_mul(out=rq[:], in0=r[:], scalar1=1.0 / HEADS)
            tmp = pool.tile([P, f2], F32, tag="tmp2")
            nc.vector.tensor_tensor(
                out=tmp[:].rearrange("p (h c) -> p h c", h=HEADS),
                in0=u[:].rearrange("p (h c) -> p h c", h=HEADS),
                in1=rq[:].to_broadcast([P, HEADS, out_ch]),
                op=mybir.AluOpType.mult,
            )
            o = pool.tile([P, out_ch], F32, tag="o2")
            nc.vector.tensor_reduce(
                out=o[:],
                in_=tmp[:].rearrange("p (h c) -> p c h", h=HEADS),
                axis=mybir.AxisListType.X,
                op=mybir.AluOpType.add,
            )
            nc.vector.tensor_add(out=o[:], in0=o[:], in1=b2_b[:])
            nc.sync.dma_start(out=out2_d[g * P : (g + 1) * P, :], in_=o[:])

        aggregate(f2, h2cat, adstp2, finish2)

    _bass_rust.generate_event_semaphores(nc)
    return nc


def kernel(x, edge_index, W1, att_src1, att_dst1, b1, W2, att_src2, att_dst2, b2, trace=False):
    x = np.asarray(x, dtype=np.float32)
    edge_index = np.asarray(edge_index)
    in_ch = x.shape[1]
    hid = np.asarray(att_src1).shape[1]
    out_ch = np.asarray(att_src2).shape[1]
    f1, f2 = HEADS * hid, HEADS * out_ch

    x_tabT, idx_arr, adstbase, meta = _host_prep(x, edge_index)
    nc = _build_program(meta, in_ch, hid, out_ch)

    common = {
        "x_tabT": x_tabT,
        "w1": np.asarray(W1, dtype=np.float32),
        "asrc1": np.asarray(att_src1, dtype=np.float32).reshape(1, f1),
        "adst1": np.asarray(att_dst1, dtype=np.float32).reshape(1, f1),
        "b1": np.asarray(b1, dtype=np.float32).reshape(1, f1),
        "w2": np.asarray(W2, dtype=np.float32),
        "asrc2": np.asarray(att_src2, dtype=np.float32).reshape(1, f2),
        "adst2": np.asarray(att_dst2, dtype=np.float32).reshape(1, f2),
        "b2": np.asarray(b2, dtype=np.float32).reshape(1, out_ch),
    }
    in_maps = [
        {**common, "idx": np.ascontiguousarray(idx_arr[c]), "adstbase": adstbase[c]}
        for c in range(NC)
    ]
    if trace:
        import axon_prof

        axon_prof.install()
    r = run_bass_kernel_spmd(nc, in_maps, list(range(NC)), trace=trace)

    n, nloc, order = meta["n"], meta["nloc"], meta["order"]
    out = np.zeros((n, out_ch), dtype=np.float32)
    for c in range(NC):
        j = np.arange(nloc)
        rk = j * NC + c
        valid = rk < n
        out[order[rk[valid]]] = r.results[c]["out2"][valid]
    if trace:
        return out, r
    return out


# revision 11
# speedup vs baseline: 1.0097x; 1.0097x over previous
"""Two-layer GAT (PyG GATConv semantics) on 8 Trainium2 NeuronCores.

Strategy (node/graph parallel):
- Nodes are degree-sorted and dealt round-robin to 8 cores (dst sharding).
- Every core computes the full [h | a_src] projection tables for both layers
  (replicated compute beats all-gathering the 100MB layer-2 table).
- Per-edge work is dst-aligned: for each group of 128 destination nodes
  (one per SBUF partition), padded per-lane slot lists drive [128,1]-indexed
  row gathers from the projection table; attention softmax runs without
  max-subtraction (logits are O(10), fp32 exp is safe; the 1e-16 eps makes
  the normalization exactly alpha = ex / (sum ex + eps) either way).
- Sentinel table row (a_src = -1000, h = 0) makes padded slots contribute
  exactly zero weight.
- Single AllGather exchanges the transposed layer-1 outputs (the only
  cross-core communication).
"""
import sys

sys.path.insert(0, "/opt/trn_rl_repo")

from contextlib import ExitStack

import numpy as np

import concourse.bass as bass
import concourse.tile as tile
from concourse import mybir
import bass_rust as _bass_rust
from concourse.bass_utils import run_bass_kernel_spmd
from concourse.masks import make_identity

NC = 8
P = 128
HEADS = 4
NEG_SLOPE = 0.2
EPS = 1e-16
SENT_ASRC = -1000.0

F32 = mybir.dt.float32
BF16 = mybir.dt.bfloat16
I32 = mybir.dt.int32
import os
TABLE_BF16 = os.environ.get("GAT_TABLE_BF16", "0") == "1"
TDT = BF16 if TABLE_BF16 else F32


def _host_prep(x, edge_index):
    n, in_ch = x.shape
    src = np.concatenate([np.asarray(edge_index[0]), np.arange(n, dtype=np.int64)])
    dst = np.concatenate([np.asarray(edge_index[1]), np.arange(n, dtype=np.int64)])
    deg = np.bincount(dst, minlength=n)

    order = np.argsort(-deg, kind="stable")  # order[rank] = node
    rank = np.empty(n, dtype=np.int64)
    rank[order] = np.arange(n)

    nloc = ((n + NC - 1) // NC + P - 1) // P * P  # local slots per core
    ntab = NC * nloc
    ng = nloc // P

    # rank r -> core r%NC, slot r//NC, table row t
    t_of = (rank % NC) * nloc + rank // NC  # per node

    x_tab = np.zeros((ntab, in_ch), dtype=np.float32)
    x_tab[t_of] = np.asarray(x, dtype=np.float32)
    x_tabT = np.ascontiguousarray(x_tab.T)  # [in_ch, ntab]

    td = t_of[dst]
    ts = t_of[src].astype(np.int32)
    c_e = td // nloc
    loc = td % nloc
    g_e = loc // P
    lane_e = loc % P

    key = (c_e * ng + g_e) * P + lane_e
    cnt = np.bincount(key, minlength=NC * ng * P).reshape(NC, ng, P)
    s_g = np.maximum(cnt.max(axis=(0, 2)), 1)  # padded slots per group
    s0 = np.zeros(ng, dtype=np.int64)
    s0[1:] = np.cumsum(s_g)[:-1]
    st = int(s_g.sum())

    sidx = np.argsort(key, kind="stable")
    ks = key[sidx]
    # slot index within each key run
    starts = np.searchsorted(ks, np.arange(NC * ng * P))
    slot = np.arange(len(ks)) - starts[ks]

    idx_arr = np.full((NC, P, st), ntab, dtype=np.int32)  # sentinel row id
    col = s0[g_e[sidx]] + slot
    idx_arr[c_e[sidx], lane_e[sidx], col] = ts[sidx]

    adstbase = np.stack(
        [(c * nloc + np.arange(P) * ng).astype(np.int32).reshape(P, 1) for c in range(NC)]
    )

    meta = {
        "n": n,
        "nloc": nloc,
        "ntab": ntab,
        "ng": ng,
        "st": st,
        "s_g": s_g.astype(np.int64),
        "s0": s0,
        "order": order,
    }
    return x_tabT, idx_arr, adstbase, meta


def _build_program(meta, in_ch, hid, out_ch):
    """One SPMD program for all 8 cores."""
    ntab, ng, st, nloc = meta["ntab"], meta["ng"], meta["st"], meta["nloc"]
    s_g, s0 = meta["s_g"], meta["s0"]
    f1 = HEADS * hid       # 64
    f2 = HEADS * out_ch    # 256
    ntiles = ntab // P

    nc = bass.Bass(num_devices=NC)

    x_tabT = nc.declare_dram_parameter("x_tabT", [in_ch, ntab], F32, isOutput=False)
    idx_d = nc.declare_dram_parameter("idx", [P, st], I32, isOutput=False)
    adstbase_d = nc.declare_dram_parameter("adstbase", [P, 1], I32, isOutput=False)
    w1_d = nc.declare_dram_parameter("w1", [in_ch, f1], F32, isOutput=False)
    asrc1_d = nc.declare_dram_parameter("asrc1", [1, f1], F32, isOutput=False)
    adst1_d = nc.declare_dram_parameter("adst1", [1, f1], F32, isOutput=False)
    b1_d = nc.declare_dram_parameter("b1", [1, f1], F32, isOutput=False)
    w2_d = nc.declare_dram_parameter("w2", [f1, f2], F32, isOutput=False)
    asrc2_d = nc.declare_dram_parameter("asrc2", [1, f2], F32, isOutput=False)
    adst2_d = nc.declare_dram_parameter("adst2", [1, f2], F32, isOutput=False)
    b2_d = nc.declare_dram_parameter("b2", [1, out_ch], F32, isOutput=False)
    out2_d = nc.declare_dram_parameter("out2", [nloc, out_ch], F32, isOutput=True)

    h1cat = nc.dram_tensor("h1cat", [ntab + 1, f1 + 4], TDT)
    adstp1 = nc.dram_tensor("adstp1", [ntab, 4], TDT)
    h2cat = nc.dram_tensor("h2cat", [ntab + 1, f2 + 4], TDT)
    adstp2 = nc.dram_tensor("adstp2", [ntab, 4], TDT)
    out1t = nc.dram_tensor("out1t", [f1, nloc], F32)
    ag_out = nc.dram_tensor("ag_out", [NC * f1, nloc], F32, addr_space="Shared")

    def bcast_row(dram_t, width):
        return bass.AP(
            tensor=dram_t[:].tensor,
            offset=dram_t[:].offset,
            ap=[[0, P], [1, width]],
        )

    with tile.TileContext(nc) as tc, ExitStack() as ctx:
        const = ctx.enter_context(tc.tile_pool(name="const", bufs=1))
        pool = ctx.enter_context(tc.tile_pool(name="work", bufs=3))
        gpool = ctx.enter_context(tc.tile_pool(name="gather", bufs=2))
        psum = ctx.enter_context(tc.tile_pool(name="psum", bufs=2, space="PSUM"))
        psum1 = ctx.enter_context(tc.tile_pool(name="psum1", bufs=1, space="PSUM"))

        # ---- constants ----
        w1_sb = const.tile([in_ch, f1], F32)
        nc.sync.dma_start(out=w1_sb[:], in_=w1_d[:, :])
        w2_sb = const.tile([f1, f2], F32)
        nc.sync.dma_start(out=w2_sb[:], in_=w2_d[:, :])
        b1_b = const.tile([P, f1], F32)
        nc.sync.dma_start(out=b1_b[:], in_=bcast_row(b1_d, f1))
        b2_b = const.tile([P, out_ch], F32)
        nc.sync.dma_start(out=b2_b[:], in_=bcast_row(b2_d, out_ch))
        ident = const.tile([P, P], F32)
        make_identity(nc, ident[:])
        idx_sb = const.tile([P, st], I32)
        nc.sync.dma_start(out=idx_sb[:], in_=idx_d[:, :])
        adstbase_sb = const.tile([P, 1], I32)
        nc.sync.dma_start(out=adstbase_sb[:], in_=adstbase_d[:, :])

        # sentinel rows
        sent1 = const.tile([1, f1 + 4], TDT)
        nc.vector.memset(sent1[:], 0.0)
        nc.vector.memset(sent1[:, f1 : f1 + 4], SENT_ASRC)
        nc.sync.dma_start(out=h1cat[ntab : ntab + 1, :], in_=sent1[:])
        sent2 = const.tile([1, f2 + 4], TDT)
        nc.vector.memset(sent2[:], 0.0)
        nc.vector.memset(sent2[:, f2 : f2 + 4], SENT_ASRC)
        nc.sync.dma_start(out=h2cat[ntab : ntab + 1, :], in_=sent2[:])

        # ---- augmented weights: Waug = [W | W @ Asrc_bd | W @ Adst_bd] ----
        # Block-diagonal attention matrices A[h*ch + c, h] = att[h, c], built by
        # DMA'ing each head's att row into a column block.
        def build_attmat(att_d, fdim, tag):
            # returns per-128-row chunks of the [fdim, 4] block-diag matrix
            ch = fdim // HEADS
            chunks = []
            for k0 in range(0, fdim, P):
                rows = min(P, fdim - k0)
                a_sb = const.tile([rows, 4], F32, tag=f"{tag}_{k0}")
                nc.vector.memset(a_sb[:], 0.0)
                for h in range(HEADS):
                    lo, hi = h * ch, (h + 1) * ch
                    lo2, hi2 = max(lo, k0), min(hi, k0 + rows)
                    if lo2 < hi2:
                        nc.sync.dma_start(
                            out=a_sb[lo2 - k0 : hi2 - k0, h : h + 1],
                            in_=att_d[0:1, lo2:hi2],
                        )
                chunks.append(a_sb)
            return chunks

        as1_m = build_attmat(asrc1_d, f1, "as1m")
        ad1_m = build_attmat(adst1_d, f1, "ad1m")
        as2_m = build_attmat(asrc2_d, f2, "as2m")
        ad2_m = build_attmat(adst2_d, f2, "ad2m")

        # W1 @ A (contraction over f1=64): lhsT = W1^T via PE transpose.
        w1aug = const.tile([in_ch, f1 + 8], F32)
        nc.vector.tensor_copy(out=w1aug[:, 0:f1], in_=w1_sb[:])
        w1t_ps = psum1.tile([f1, in_ch], F32, tag="prep_t")
        nc.tensor.transpose(out=w1t_ps[:], in_=w1_sb[:], identity=ident[:])
        w1t = const.tile([f1, in_ch], F32)
        nc.vector.tensor_copy(out=w1t[:], in_=w1t_ps[:])
        w1as_ps = psum1.tile([in_ch, 4], F32, tag="prep_a")
        w1ad_ps = psum1.tile([in_ch, 4], F32, tag="prep_b")
        nc.tensor.matmul(out=w1as_ps[:], lhsT=w1t[:], rhs=as1_m[0][:], start=True, stop=True)
        nc.tensor.matmul(out=w1ad_ps[:], lhsT=w1t[:], rhs=ad1_m[0][:], start=True, stop=True)
        nc.vector.tensor_copy(out=w1aug[:, f1 : f1 + 4], in_=w1as_ps[:])
        nc.vector.tensor_copy(out=w1aug[:, f1 + 4 : f1 + 8], in_=w1ad_ps[:])

        # W2 @ A2 (contraction over f2=256, split into K=128 halves).
        w2aug = const.tile([f1, f2 + 8], F32)
        nc.vector.tensor_copy(out=w2aug[:, 0:f2], in_=w2_sb[:])
        w2as_ps = psum1.tile([f1, 4], F32, tag="prep_a")
        w2ad_ps = psum1.tile([f1, 4], F32, tag="prep_b")
        nkh = f2 // P
        for kh in range(nkh):
            w2t_ps = psum1.tile([P, f1], F32, tag="prep_t")
            nc.tensor.transpose(
                out=w2t_ps[:], in_=w2_sb[:, kh * P : (kh + 1) * P],
                identity=ident[0:f1, 0:f1],
            )
            w2t = pool.tile([P, f1], F32, tag="w2t_sb")
            nc.vector.tensor_copy(out=w2t[:], in_=w2t_ps[:])
            nc.tensor.matmul(
                out=w2as_ps[:], lhsT=w2t[:], rhs=as2_m[kh][:],
                start=(kh == 0), stop=(kh == nkh - 1),
            )
            nc.tensor.matmul(
                out=w2ad_ps[:], lhsT=w2t[:], rhs=ad2_m[kh][:],
                start=(kh == 0), stop=(kh == nkh - 1),
            )
        nc.vector.tensor_copy(out=w2aug[:, f2 : f2 + 4], in_=w2as_ps[:])
        nc.vector.tensor_copy(out=w2aug[:, f2 + 4 : f2 + 8], in_=w2ad_ps[:])

        def projection(t_i, lhsT_ap, waug_sb, fdim, hcat, adstp):
            ps = psum.tile([P, fdim + 8], F32, tag="proj_ps")
            nc.tensor.matmul(out=ps[:], lhsT=lhsT_ap, rhs=waug_sb[:], start=True, stop=True)
            hc = pool.tile([P, fdim + 8], TDT, tag="proj_hc")
            nc.vector.tensor_copy(out=hc[:], in_=ps[:])
            nc.sync.dma_start(
                out=hcat[t_i * P : (t_i + 1) * P, :], in_=hc[:, 0 : fdim + 4]
            )
            c_i, g_i = t_i // ng, t_i % ng
            nc.sync.dma_start(
                out=adstp[:].rearrange("(c l g) f -> c l g f", c=NC, l=P)[c_i, :, g_i, :],
                in_=hc[:, fdim + 4 : fdim + 8],
            )

        # ---- P1: layer-1 projection table (all nodes, replicated) ----
        for t_i in range(ntiles):
            xt = pool.tile([in_ch, P], F32, tag="p1_x")
            nc.sync.dma_start(out=xt[:], in_=x_tabT[:, t_i * P : (t_i + 1) * P])
            projection(t_i, xt[:], w1aug, f1, h1cat, adstp1)

        def aggregate(fdim, hcat, adstp, finish):
            """Per dst-group attention + weighted aggregation."""
            adst_own = const.tile([P, ng, 4], TDT, tag=f"adst_own{fdim}")
            nc.gpsimd.indirect_dma_start(
                out=adst_own[:].rearrange("p g f -> p (g f)"),
                out_offset=None,
                in_=adstp[:],
                in_offset=bass.IndirectOffsetOnAxis(ap=adstbase_sb[:], axis=0),
            )
            smax = int(s_g.max())
            for g in range(ng):
                sg = int(s_g[g])
                base = int(s0[g])
                t_sb = gpool.tile([P, smax, fdim + 4], TDT, tag=f"t{fdim}")
                for j in range(sg):
                    nc.gpsimd.indirect_dma_start(
                        out=t_sb[:, j, :],
                        out_offset=None,
                        in_=hcat[:],
                        in_offset=bass.IndirectOffsetOnAxis(
                            ap=idx_sb[:, base + j : base + j + 1], axis=0
                        ),
                    )
                w_sb = pool.tile([P, smax, 4], TDT, tag=f"w{fdim}")
                den = pool.tile([P, 4], F32, tag=f"den{fdim}")
                adst_g = adst_own[:, g, :]
                adst_bc = bass.AP(
                    tensor=adst_g.tensor,
                    offset=adst_g.offset,
                    ap=[adst_g.ap[0], [0, sg], adst_g.ap[1]],
                )
                nc.vector.tensor_tensor(
                    out=w_sb[:, :sg, :],
                    in0=t_sb[:, :sg, fdim : fdim + 4],
                    in1=adst_bc,
                    op=mybir.AluOpType.add,
                )
                nc.vector.scalar_tensor_tensor(
                    out=w_sb[:, :sg, :],
                    in0=w_sb[:, :sg, :],
                    scalar=NEG_SLOPE,
                    in1=w_sb[:, :sg, :],
                    op0=mybir.AluOpType.mult,
                    op1=mybir.AluOpType.max,
                )
                for h in range(HEADS):
                    nc.scalar.activation(
                        out=w_sb[:, :sg, h],
                        in_=w_sb[:, :sg, h],
                        func=mybir.ActivationFunctionType.Exp,
                        accum_out=den[:, h : h + 1],
                    )
                hv = t_sb[:, :sg, 0:fdim].rearrange("p g (h c) -> p g h c", h=HEADS)
                nc.vector.tensor_tensor(
                    out=hv,
                    in0=hv,
                    in1=w_sb[:, :sg, :].to_broadcast([P, sg, HEADS, fdim // HEADS]),
                    op=mybir.AluOpType.mult,
                )
                u = pool.tile([P, fdim], F32, tag=f"u{fdim}")
                nc.vector.tensor_reduce(
                    out=u[:],
                    in_=t_sb[:, :sg, 0:fdim].rearrange("p g f -> p f g"),
                    axis=mybir.AxisListType.X,
                    op=mybir.AluOpType.add,
                )
                r = pool.tile([P, 4], F32, tag=f"r{fdim}")
                nc.vector.tensor_scalar_add(out=r[:], in0=den[:], scalar1=EPS)
                nc.vector.reciprocal(out=r[:], in_=r[:])
                finish(g, u, r)

        # ---- A1: layer-1 aggregation -> out1t (transposed, for AG) ----
        def finish1(g, u, r):
            o = pool.tile([P, f1], F32, tag="o1")
            nc.vector.tensor_tensor(
                out=o[:].rearrange("p (h c) -> p h c", h=HEADS),
                in0=u[:].rearrange("p (h c) -> p h c", h=HEADS),
                in1=r[:].to_broadcast([P, HEADS, f1 // HEADS]),
                op=mybir.AluOpType.mult,
            )
            nc.vector.tensor_add(out=o[:], in0=o[:], in1=b1_b[:])
            nc.vector.tensor_scalar_max(out=o[:], in0=o[:], scalar1=0.0)
            tps = psum.tile([f1, P], F32, tag="tr_ps")
            nc.tensor.transpose(out=tps[:], in_=o[:], identity=ident[:])
            ot = pool.tile([f1, P], F32, tag="o1t")
            nc.vector.tensor_copy(out=ot[:], in_=tps[:])
            nc.sync.dma_start(out=out1t[:, g * P : (g + 1) * P], in_=ot[:])

        aggregate(f1, h1cat, adstp1, finish1)

        # ---- AG: exchange layer-1 outputs ----
        nc.gpsimd.collective_compute(
            "AllGather",
            mybir.AluOpType.bypass,
            replica_groups=[list(range(NC))],
            ins=[out1t[:]],
            outs=[ag_out[:]],
        )

        # ---- P2: layer-2 projection table ----
        for t_i in range(ntiles):
            c_i, g_i = t_i // ng, t_i % ng
            lt = pool.tile([f1, P], F32, tag="p2_l")
            nc.sync.dma_start(
                out=lt[:], in_=ag_out[c_i * f1 : (c_i + 1) * f1, g_i * P : (g_i + 1) * P]
            )
            projection(t_i, lt[:], w2aug, f2, h2cat, adstp2)

        # ---- A2: layer-2 aggregation -> mean over heads + b2 ----
        def finish2(g, u, r):
            rq = pool.tile([P, 4], F32, tag="rq")
            nc.vector.tensor_scalar_mul(out=rq[:], in0=r[:], scalar1=1.0 / HEADS)
            tmp = pool.tile([P, f2], F32, tag="tmp2")
            nc.vector.tensor_tensor(
                out=tmp[:].rearrange("p (h c) -> p h c", h=HEADS),
                in0=u[:].rearrange("p (h c) -> p h c", h=HEADS),
                in1=rq[:].to_broadcast([P, HEADS, out_ch]),
                op=mybir.AluOpType.mult,
            )
            o = pool.tile([P, out_ch], F32, tag="o2")
            nc.vector.tensor_reduce(
                out=o[:],
                in_=tmp[:].rearrange("p (h c) -> p c h", h=HEADS),
                axis=mybir.AxisListType.X,
                op=mybir.AluOpType.add,
            )
            nc.vector.tensor_add(out=o[:], in0=o[:], in1=b2_b[:])
            nc.sync.dma_start(out=out2_d[g * P : (g + 1) * P, :], in_=o[:])

        aggregate(f2, h2cat, adstp2, finish2)

    _bass_rust.generate_event_semaphores(nc)
    return nc


def kernel(x, edge_index, W1, att_src1, att_dst1, b1, W2, att_src2, att_dst2, b2, trace=False):
    x = np.asarray(x, dtype=np.float32)
    edge_index = np.asarray(edge_index)
    in_ch = x.shape[1]
    hid = np.asarray(att_src1).shape[1]
    out_ch = np.asarray(att_src2).shape[1]
    f1, f2 = HEADS * hid, HEADS * out_ch

    x_tabT, idx_arr, adstbase, meta = _host_prep(x, edge_index)
    nc = _build_program(meta, in_ch, hid, out_ch)

    common = {
        "x_tabT": x_tabT,
        "w1": np.asarray(W1, dtype=np.float32),
        "asrc1": np.asarray(att_src1, dtype=np.float32).reshape(1, f1),
        "adst1": np.asarray(att_dst1, dtype=np.float32).reshape(1, f1),
        "b1": np.asarray(b1, dtype=np.float32).reshape(1, f1),
        "w2": np.asarray(W2, dtype=np.float32),
        "asrc2": np.asarray(att_src2, dtype=np.float32).reshape(1, f2),
        "adst2": np.asarray(att_dst2, dtype=np.float32).reshape(1, f2),
        "b2": np.asarray(b2, dtype=np.float32).reshape(1, out_ch),
    }
    in_maps = [
        {**common, "idx": np.ascontiguousarray(idx_arr[c]), "adstbase": adstbase[c]}
        for c in range(NC)
    ]
    if trace:
        import axon_prof

        axon_prof.install()
    r = run_bass_kernel_spmd(nc, in_maps, list(range(NC)), trace=trace)

    n, nloc, order = meta["n"], meta["nloc"], meta["order"]
    out = np.zeros((n, out_ch), dtype=np.float32)
    for c in range(NC):
        j = np.arange(nloc)
        rk = j * NC + c
        valid = rk < n
        out[order[rk[valid]]] = r.results[c]["out2"][valid]
    if trace:
        return out, r
    return out


# revision 15
# speedup vs baseline: 1.1163x; 1.1056x over previous
"""Two-layer GAT (PyG GATConv semantics) on 8 Trainium2 NeuronCores.

Strategy (node/graph parallel):
- Nodes are degree-sorted and dealt round-robin to 8 cores (dst sharding).
- Every core computes the full [h | a_src] projection tables for both layers
  (replicated compute beats all-gathering the 100MB layer-2 table).
- Per-edge work is dst-aligned: for each group of 128 destination nodes
  (one per SBUF partition), padded per-lane slot lists drive [128,1]-indexed
  row gathers from the projection table; attention softmax runs without
  max-subtraction (logits are O(10), fp32 exp is safe; the 1e-16 eps makes
  the normalization exactly alpha = ex / (sum ex + eps) either way).
- Sentinel table row (a_src = -1000, h = 0) makes padded slots contribute
  exactly zero weight.
- Single AllGather exchanges the transposed layer-1 outputs (the only
  cross-core communication).
"""
import sys

sys.path.insert(0, "/opt/trn_rl_repo")

from contextlib import ExitStack

import numpy as np

import concourse.bass as bass
import concourse.tile as tile
from concourse import mybir
import bass_rust as _bass_rust
from concourse.bass_utils import run_bass_kernel_spmd
from concourse.masks import make_identity

NC = 8
P = 128
HEADS = 4
NEG_SLOPE = 0.2
EPS = 1e-16
SENT_ASRC = -1000.0

F32 = mybir.dt.float32
BF16 = mybir.dt.bfloat16
I32 = mybir.dt.int32
import os
TABLE_BF16 = os.environ.get("GAT_TABLE_BF16", "0") == "1"
TDT = BF16 if TABLE_BF16 else F32


def _host_prep(x, edge_index):
    n, in_ch = x.shape
    src = np.concatenate([np.asarray(edge_index[0]), np.arange(n, dtype=np.int64)])
    dst = np.concatenate([np.asarray(edge_index[1]), np.arange(n, dtype=np.int64)])
    deg = np.bincount(dst, minlength=n)

    order = np.argsort(-deg, kind="stable")  # order[rank] = node
    rank = np.empty(n, dtype=np.int64)
    rank[order] = np.arange(n)

    nloc = ((n + NC - 1) // NC + P - 1) // P * P  # local slots per core
    ntab = NC * nloc
    ng = nloc // P

    # rank r -> core r%NC, slot r//NC, table row t
    t_of = (rank % NC) * nloc + rank // NC  # per node

    x_tab = np.zeros((ntab, in_ch), dtype=np.float32)
    x_tab[t_of] = np.asarray(x, dtype=np.float32)
    x_tabT = np.ascontiguousarray(x_tab.T)  # [in_ch, ntab]

    td = t_of[dst]
    ts = t_of[src].astype(np.int32)
    c_e = td // nloc
    loc = td % nloc
    g_e = loc // P
    lane_e = loc % P

    key = (c_e * ng + g_e) * P + lane_e
    cnt = np.bincount(key, minlength=NC * ng * P).reshape(NC, ng, P)
    s_g = np.maximum(cnt.max(axis=(0, 2)), 1)  # padded slots per group
    s0 = np.zeros(ng, dtype=np.int64)
    s0[1:] = np.cumsum(s_g)[:-1]
    st = int(s_g.sum())

    sidx = np.argsort(key, kind="stable")
    ks = key[sidx]
    # slot index within each key run
    starts = np.searchsorted(ks, np.arange(NC * ng * P))
    slot = np.arange(len(ks)) - starts[ks]

    idx_arr = np.full((NC, P, st), ntab, dtype=np.int32)  # sentinel row id
    col = s0[g_e[sidx]] + slot
    idx_arr[c_e[sidx], lane_e[sidx], col] = ts[sidx]

    adstbase = np.stack(
        [(c * nloc + np.arange(P) * ng).astype(np.int32).reshape(P, 1) for c in range(NC)]
    )

    meta = {
        "n": n,
        "nloc": nloc,
        "ntab": ntab,
        "ng": ng,
        "st": st,
        "s_g": s_g.astype(np.int64),
        "s0": s0,
        "order": order,
    }
    return x_tabT, idx_arr, adstbase, meta


def _build_program(meta, in_ch, hid, out_ch):
    """One SPMD program for all 8 cores."""
    ntab, ng, st, nloc = meta["ntab"], meta["ng"], meta["st"], meta["nloc"]
    s_g, s0 = meta["s_g"], meta["s0"]
    f1 = HEADS * hid       # 64
    f2 = HEADS * out_ch    # 256
    ntiles = ntab // P

    nc = bass.Bass(num_devices=NC)

    x_tabT = nc.declare_dram_parameter("x_tabT", [in_ch, ntab], F32, isOutput=False)
    idx_d = nc.declare_dram_parameter("idx", [P, st], I32, isOutput=False)
    adstbase_d = nc.declare_dram_parameter("adstbase", [P, 1], I32, isOutput=False)
    w1_d = nc.declare_dram_parameter("w1", [in_ch, f1], F32, isOutput=False)
    asrc1_d = nc.declare_dram_parameter("asrc1", [1, f1], F32, isOutput=False)
    adst1_d = nc.declare_dram_parameter("adst1", [1, f1], F32, isOutput=False)
    b1_d = nc.declare_dram_parameter("b1", [1, f1], F32, isOutput=False)
    w2_d = nc.declare_dram_parameter("w2", [f1, f2], F32, isOutput=False)
    asrc2_d = nc.declare_dram_parameter("asrc2", [1, f2], F32, isOutput=False)
    adst2_d = nc.declare_dram_parameter("adst2", [1, f2], F32, isOutput=False)
    b2_d = nc.declare_dram_parameter("b2", [1, out_ch], F32, isOutput=False)
    out2_d = nc.declare_dram_parameter("out2", [nloc, out_ch], F32, isOutput=True)

    h1cat = nc.dram_tensor("h1cat", [ntab + 1, f1 + 4], TDT)
    adstp1 = nc.dram_tensor("adstp1", [ntab, 4], TDT)
    h2cat = nc.dram_tensor("h2cat", [ntab + 1, f2 + 4], TDT)
    adstp2 = nc.dram_tensor("adstp2", [ntab, 4], TDT)
    out1t = nc.dram_tensor("out1t", [f1, nloc], F32)
    ag_out = nc.dram_tensor("ag_out", [NC * f1, nloc], F32, addr_space="Shared")

    def bcast_row(dram_t, width):
        return bass.AP(
            tensor=dram_t[:].tensor,
            offset=dram_t[:].offset,
            ap=[[0, P], [1, width]],
        )

    with tile.TileContext(nc) as tc, ExitStack() as ctx:
        const = ctx.enter_context(tc.tile_pool(name="const", bufs=1))
        pool = ctx.enter_context(tc.tile_pool(name="work", bufs=3))
        gpool = ctx.enter_context(tc.tile_pool(name="gather", bufs=2))
        psum1_cm = tc.tile_pool(name="psum1", bufs=1, space="PSUM")
        psum1 = psum1_cm.__enter__()

        # ---- constants ----
        w1_sb = const.tile([in_ch, f1], F32)
        nc.sync.dma_start(out=w1_sb[:], in_=w1_d[:, :])
        w2_sb = const.tile([f1, f2], F32)
        nc.sync.dma_start(out=w2_sb[:], in_=w2_d[:, :])
        b1_b = const.tile([P, f1], F32)
        nc.sync.dma_start(out=b1_b[:], in_=bcast_row(b1_d, f1))
        b2_b = const.tile([P, out_ch], F32)
        nc.sync.dma_start(out=b2_b[:], in_=bcast_row(b2_d, out_ch))
        ident = const.tile([P, P], F32)
        make_identity(nc, ident[:])
        idx_sb = const.tile([P, st], I32)
        nc.sync.dma_start(out=idx_sb[:], in_=idx_d[:, :])
        adstbase_sb = const.tile([P, 1], I32)
        nc.sync.dma_start(out=adstbase_sb[:], in_=adstbase_d[:, :])

        # sentinel rows
        sent1 = const.tile([1, f1 + 4], TDT)
        nc.vector.memset(sent1[:], 0.0)
        nc.vector.memset(sent1[:, f1 : f1 + 4], SENT_ASRC)
        nc.sync.dma_start(out=h1cat[ntab : ntab + 1, :], in_=sent1[:])
        sent2 = const.tile([1, f2 + 4], TDT)
        nc.vector.memset(sent2[:], 0.0)
        nc.vector.memset(sent2[:, f2 : f2 + 4], SENT_ASRC)
        nc.sync.dma_start(out=h2cat[ntab : ntab + 1, :], in_=sent2[:])

        # ---- augmented weights: Waug = [W | W @ Asrc_bd | W @ Adst_bd] ----
        # Block-diagonal attention matrices A[h*ch + c, h] = att[h, c], built by
        # DMA'ing each head's att row into a column block.
        def build_attmat(att_d, fdim, tag):
            # returns per-128-row chunks of the [fdim, 4] block-diag matrix
            ch = fdim // HEADS
            chunks = []
            for k0 in range(0, fdim, P):
                rows = min(P, fdim - k0)
                a_sb = const.tile([rows, 4], F32, tag=f"{tag}_{k0}")
                nc.vector.memset(a_sb[:], 0.0)
                for h in range(HEADS):
                    lo, hi = h * ch, (h + 1) * ch
                    lo2, hi2 = max(lo, k0), min(hi, k0 + rows)
                    if lo2 < hi2:
                        nc.sync.dma_start(
                            out=a_sb[lo2 - k0 : hi2 - k0, h : h + 1],
                            in_=att_d[0:1, lo2:hi2],
                        )
                chunks.append(a_sb)
            return chunks

        as1_m = build_attmat(asrc1_d, f1, "as1m")
        ad1_m = build_attmat(adst1_d, f1, "ad1m")
        as2_m = build_attmat(asrc2_d, f2, "as2m")
        ad2_m = build_attmat(adst2_d, f2, "ad2m")

        # W1 @ A (contraction over f1=64): lhsT = W1^T via PE transpose.
        w1aug = const.tile([in_ch, f1 + 8], F32)
        nc.vector.tensor_copy(out=w1aug[:, 0:f1], in_=w1_sb[:])
        w1t_ps = psum1.tile([f1, in_ch], F32, tag="prep_t")
        nc.tensor.transpose(out=w1t_ps[:], in_=w1_sb[:], identity=ident[:])
        w1t = const.tile([f1, in_ch], F32)
        nc.vector.tensor_copy(out=w1t[:], in_=w1t_ps[:])
        w1as_ps = psum1.tile([in_ch, 4], F32, tag="prep_a")
        w1ad_ps = psum1.tile([in_ch, 4], F32, tag="prep_b")
        nc.tensor.matmul(out=w1as_ps[:], lhsT=w1t[:], rhs=as1_m[0][:], start=True, stop=True)
        nc.tensor.matmul(out=w1ad_ps[:], lhsT=w1t[:], rhs=ad1_m[0][:], start=True, stop=True)
        nc.vector.tensor_copy(out=w1aug[:, f1 : f1 + 4], in_=w1as_ps[:])
        nc.vector.tensor_copy(out=w1aug[:, f1 + 4 : f1 + 8], in_=w1ad_ps[:])

        # W2 @ A2 (contraction over f2=256, split into K=128 halves).
        w2aug = const.tile([f1, f2 + 8], F32)
        nc.vector.tensor_copy(out=w2aug[:, 0:f2], in_=w2_sb[:])
        w2as_ps = psum1.tile([f1, 4], F32, tag="prep_a")
        w2ad_ps = psum1.tile([f1, 4], F32, tag="prep_b")
        nkh = f2 // P
        for kh in range(nkh):
            w2t_ps = psum1.tile([P, f1], F32, tag="prep_t")
            nc.tensor.transpose(
                out=w2t_ps[:], in_=w2_sb[:, kh * P : (kh + 1) * P],
                identity=ident[0:f1, 0:f1],
            )
            w2t = pool.tile([P, f1], F32, tag="w2t_sb")
            nc.vector.tensor_copy(out=w2t[:], in_=w2t_ps[:])
            nc.tensor.matmul(
                out=w2as_ps[:], lhsT=w2t[:], rhs=as2_m[kh][:],
                start=(kh == 0), stop=(kh == nkh - 1),
            )
            nc.tensor.matmul(
                out=w2ad_ps[:], lhsT=w2t[:], rhs=ad2_m[kh][:],
                start=(kh == 0), stop=(kh == nkh - 1),
            )
        nc.vector.tensor_copy(out=w2aug[:, f2 : f2 + 4], in_=w2as_ps[:])
        nc.vector.tensor_copy(out=w2aug[:, f2 + 4 : f2 + 8], in_=w2ad_ps[:])
        psum1_cm.__exit__(None, None, None)
        psum = ctx.enter_context(tc.tile_pool(name="psum", bufs=5, space="PSUM"))
        psumt = ctx.enter_context(tc.tile_pool(name="psumt", bufs=2, space="PSUM"))

        QMAX = 4

        def projection_quad(c_i, g0, q, lhsT_tile, waug_sb, fdim, hcat, adstp):
            hc = pool.tile([P, QMAX, fdim + 8], TDT, tag="proj_hc")
            for k in range(q):
                ps = psum.tile([P, fdim + 8], F32, tag="proj_ps")
                nc.tensor.matmul(
                    out=ps[:], lhsT=lhsT_tile[:, k * P : (k + 1) * P],
                    rhs=waug_sb[:], start=True, stop=True,
                )
                nc.vector.tensor_copy(out=hc[:, k, :], in_=ps[:])
            r0 = (c_i * ng + g0) * P
            nc.sync.dma_start(
                out=hcat[r0 : r0 + q * P, :].rearrange("(k p) f -> p k f", k=q),
                in_=hc[:, :q, 0 : fdim + 4],
            )
            nc.sync.dma_start(
                out=adstp[:].rearrange("(c l g) f -> c l g f", c=NC, l=P)[
                    c_i, :, g0 : g0 + q, :
                ],
                in_=hc[:, :q, fdim + 4 : fdim + 8],
            )

        def quads():
            for c_i in range(NC):
                g0 = 0
                while g0 < ng:
                    q = min(QMAX, ng - g0)
                    yield c_i, g0, q
                    g0 += q

        # ---- P1: layer-1 projection table (all nodes, replicated) ----
        for c_i, g0, q in quads():
            xt = pool.tile([in_ch, QMAX * P], F32, tag="p1_x")
            col = (c_i * ng + g0) * P
            nc.sync.dma_start(out=xt[:, : q * P], in_=x_tabT[:, col : col + q * P])
            projection_quad(c_i, g0, q, xt, w1aug, f1, h1cat, adstp1)

        def aggregate(fdim, hcat, adstp, finish):
            """Per dst-group attention + weighted aggregation."""
            adst_own = const.tile([P, ng, 4], TDT, tag=f"adst_own{fdim}")
            nc.gpsimd.indirect_dma_start(
                out=adst_own[:].rearrange("p g f -> p (g f)"),
                out_offset=None,
                in_=adstp[:],
                in_offset=bass.IndirectOffsetOnAxis(ap=adstbase_sb[:], axis=0),
            )
            smax = int(s_g.max())
            for g in range(ng):
                sg = int(s_g[g])
                base = int(s0[g])
                t_sb = gpool.tile([P, smax, fdim + 4], TDT, tag=f"t{fdim}")
                for j in range(sg):
                    nc.gpsimd.indirect_dma_start(
                        out=t_sb[:, j, :],
                        out_offset=None,
                        in_=hcat[:],
                        in_offset=bass.IndirectOffsetOnAxis(
                            ap=idx_sb[:, base + j : base + j + 1], axis=0
                        ),
                    )
                w_sb = pool.tile([P, smax, 4], TDT, tag=f"w{fdim}")
                den = pool.tile([P, 4], F32, tag=f"den{fdim}")
                adst_g = adst_own[:, g, :]
                adst_bc = bass.AP(
                    tensor=adst_g.tensor,
                    offset=adst_g.offset,
                    ap=[adst_g.ap[0], [0, sg], adst_g.ap[1]],
                )
                nc.vector.tensor_tensor(
                    out=w_sb[:, :sg, :],
                    in0=t_sb[:, :sg, fdim : fdim + 4],
                    in1=adst_bc,
                    op=mybir.AluOpType.add,
                )
                nc.vector.scalar_tensor_tensor(
                    out=w_sb[:, :sg, :],
                    in0=w_sb[:, :sg, :],
                    scalar=NEG_SLOPE,
                    in1=w_sb[:, :sg, :],
                    op0=mybir.AluOpType.mult,
                    op1=mybir.AluOpType.max,
                )
                for h in range(HEADS):
                    nc.scalar.activation(
                        out=w_sb[:, :sg, h],
                        in_=w_sb[:, :sg, h],
                        func=mybir.ActivationFunctionType.Exp,
                        accum_out=den[:, h : h + 1],
                    )
                hv = t_sb[:, :sg, 0:fdim].rearrange("p g (h c) -> p g h c", h=HEADS)
                nc.vector.tensor_tensor(
                    out=hv,
                    in0=hv,
                    in1=w_sb[:, :sg, :].to_broadcast([P, sg, HEADS, fdim // HEADS]),
                    op=mybir.AluOpType.mult,
                )
                u = pool.tile([P, fdim], F32, tag=f"u{fdim}")
                nc.vector.tensor_reduce(
                    out=u[:],
                    in_=t_sb[:, :sg, 0:fdim].rearrange("p g f -> p f g"),
                    axis=mybir.AxisListType.X,
                    op=mybir.AluOpType.add,
                )
                r = pool.tile([P, 4], F32, tag=f"r{fdim}")
                nc.vector.tensor_scalar_add(out=r[:], in0=den[:], scalar1=EPS)
                nc.vector.reciprocal(out=r[:], in_=r[:])
                finish(g, u, r)

        # ---- A1: layer-1 aggregation -> out1t (transposed, for AG) ----
        def finish1(g, u, r):
            o = pool.tile([P, f1], F32, tag="o1")
            nc.vector.tensor_tensor(
                out=o[:].rearrange("p (h c) -> p h c", h=HEADS),
                in0=u[:].rearrange("p (h c) -> p h c", h=HEADS),
                in1=r[:].to_broadcast([P, HEADS, f1 // HEADS]),
                op=mybir.AluOpType.mult,
            )
            nc.vector.tensor_add(out=o[:], in0=o[:], in1=b1_b[:])
            nc.vector.tensor_scalar_max(out=o[:], in0=o[:], scalar1=0.0)
            tps = psumt.tile([f1, P], F32, tag="tr_ps")
            nc.tensor.transpose(out=tps[:], in_=o[:], identity=ident[:])
            ot = pool.tile([f1, P], F32, tag="o1t")
            nc.vector.tensor_copy(out=ot[:], in_=tps[:])
            nc.sync.dma_start(out=out1t[:, g * P : (g + 1) * P], in_=ot[:])

        aggregate(f1, h1cat, adstp1, finish1)

        # ---- AG: exchange layer-1 outputs ----
        nc.gpsimd.collective_compute(
            "AllGather",
            mybir.AluOpType.bypass,
            replica_groups=[list(range(NC))],
            ins=[out1t[:]],
            outs=[ag_out[:]],
        )

        # ---- P2: layer-2 projection table ----
        for c_i, g0, q in quads():
            lt = pool.tile([f1, QMAX * P], F32, tag="p2_l")
            nc.sync.dma_start(
                out=lt[:, : q * P],
                in_=ag_out[c_i * f1 : (c_i + 1) * f1, g0 * P : (g0 + q) * P],
            )
            projection_quad(c_i, g0, q, lt, w2aug, f2, h2cat, adstp2)

        # ---- A2: layer-2 aggregation -> mean over heads + b2 ----
        def finish2(g, u, r):
            rq = pool.tile([P, 4], F32, tag="rq")
            nc.vector.tensor_scalar_mul(out=rq[:], in0=r[:], scalar1=1.0 / HEADS)
            tmp = pool.tile([P, f2], F32, tag="tmp2")
            nc.vector.tensor_tensor(
                out=tmp[:].rearrange("p (h c) -> p h c", h=HEADS),
                in0=u[:].rearrange("p (h c) -> p h c", h=HEADS),
                in1=rq[:].to_broadcast([P, HEADS, out_ch]),
                op=mybir.AluOpType.mult,
            )
            o = pool.tile([P, out_ch], F32, tag="o2")
            nc.vector.tensor_reduce(
                out=o[:],
                in_=tmp[:].rearrange("p (h c) -> p c h", h=HEADS),
                axis=mybir.AxisListType.X,
                op=mybir.AluOpType.add,
            )
            nc.vector.tensor_add(out=o[:], in0=o[:], in1=b2_b[:])
            nc.sync.dma_start(out=out2_d[g * P : (g + 1) * P, :], in_=o[:])

        aggregate(f2, h2cat, adstp2, finish2)

    _bass_rust.generate_event_semaphores(nc)
    return nc


def kernel(x, edge_index, W1, att_src1, att_dst1, b1, W2, att_src2, att_dst2, b2, trace=False):
    x = np.asarray(x, dtype=np.float32)
    edge_index = np.asarray(edge_index)
    in_ch = x.shape[1]
    hid = np.asarray(att_src1).shape[1]
    out_ch = np.asarray(att_src2).shape[1]
    f1, f2 = HEADS * hid, HEADS * out_ch

    x_tabT, idx_arr, adstbase, meta = _host_prep(x, edge_index)
    nc = _build_program(meta, in_ch, hid, out_ch)

    common = {
        "x_tabT": x_tabT,
        "w1": np.asarray(W1, dtype=np.float32),
        "asrc1": np.asarray(att_src1, dtype=np.float32).reshape(1, f1),
        "adst1": np.asarray(att_dst1, dtype=np.float32).reshape(1, f1),
        "b1": np.asarray(b1, dtype=np.float32).reshape(1, f1),
        "w2": np.asarray(W2, dtype=np.float32),
        "asrc2": np.asarray(att_src2, dtype=np.float32).reshape(1, f2),
        "adst2": np.asarray(att_dst2, dtype=np.float32).reshape(1, f2),
        "b2": np.asarray(b2, dtype=np.float32).reshape(1, out_ch),
    }
    in_maps = [
        {**common, "idx": np.ascontiguousarray(idx_arr[c]), "adstbase": adstbase[c]}
        for c in range(NC)
    ]
    if trace:
        import axon_prof

        axon_prof.install()
    r = run_bass_kernel_spmd(nc, in_maps, list(range(NC)), trace=trace)

    n, nloc, order = meta["n"], meta["nloc"], meta["order"]
    out = np.zeros((n, out_ch), dtype=np.float32)
    for c in range(NC):
        j = np.arange(nloc)
        rk = j * NC + c
        valid = rk < n
        out[order[rk[valid]]] = r.results[c]["out2"][valid]
    if trace:
        return out, r
    return out
